# revision 49
# baseline (speedup 1.0000x reference)
"""VQ-VAE decoder (vq_codebook) on 8 TRN2 NeuronCores, batch-sharded.

Pipeline per core (2 batch elements = 2048 tokens):
  1. VQ: scores = 2*ze.c - ||c||^2 via bf16 matmul (n2 folded in as two extra
     bf16 contraction rows), fp16 scores -> DVE max/max_index top-8, exact
     fp32 rescore of top-4 candidates (paired-row gather + affine_mul_reduce),
     gather winning codebook row, add pos_emb.
  2. 4 transformer blocks (pre-LN MHA + pre-LN FFN), bf16 matmuls:
     - LN stats on DVE (bn_stats/bn_aggr), rstd via exp(-0.5*ln(var+eps)),
       LN gamma/beta folded into the following weights host-side.
     - h -> hT via PE transposes; qkvT layout (features on partitions).
     - Attention: scoresT orientation, 4-head row-tiled QK matmuls, ACT exp
       (softmax max-subtraction skipped; scores are O(1)), fused PV+rowsum
       matmuls via V-weights augmented with ones columns ([v|1] 64-wide
       lhsT -> psum bands [dims, sums]), normalize via base-shifted
       reciprocal + mult. V bias folded into Wo bias host-side.
  3. Output projection x @ Wout + bout -> logits fp32 (fp32 transpose of the
     residual, 2-tile staged output DMAs to amortize SP-queue issue cost).

Self-contained: hardcodes all shapes; host preps transposed/augmented/bf16
operands and LN-folded weights; runs SPMD on cores 0-7 and concatenates.
"""

import os
import numpy as np
import ml_dtypes

import concourse.bass as bass
import concourse.bacc as bacc
import concourse.mybir as mybir
from concourse import tile
from concourse import bass_utils

F32 = mybir.dt.float32
F16 = mybir.dt.float16
BF16 = mybir.dt.bfloat16
U32 = mybir.dt.uint32
U16 = mybir.dt.uint16
AF = mybir.ActivationFunctionType
ALU = mybir.AluOpType

D_MODEL, D_PATCH, K_CODES, SEQLEN = 256, 768, 8192, 1024
N_HEADS, N_BLOCKS, DFF = 8, 4, 1024
BATCH, N_CORES = 16, 8
LN_EPS = 1e-5
B_PER_CORE = BATCH // N_CORES            # 2
TOKS = B_PER_CORE * SEQLEN               # 2048
NT = TOKS // 128                         # 16 token tiles
TPB = SEQLEN // 128                      # 8 token tiles per batch element
NCAND = 8                                # exact-rescore candidates (4 pos x 2 halves)
CAUG = 264                               # padded augmented row width (256+1 pad to 8)

_BUILD_CACHE = {}


def _bf16(x):
    return np.asarray(x, np.float32).astype(ml_dtypes.bfloat16)


def build_nc(variant=None, reps=1):
    variant = variant or os.environ.get("KVAR", "full")
    do_vq = variant in ("full", "noblocks")
    do_blocks = variant in ("full", "novq")
    nc = bacc.Bacc("TRN2", target_bir_lowering=False)

    # ---------------- DRAM I/O ----------------
    zet16 = nc.dram_tensor("zet16", [258, TOKS], BF16, kind="ExternalInput")
    zeaug = nc.dram_tensor("zeaug", [TOKS, CAUG], F32, kind="ExternalInput")
    cbt16 = nc.dram_tensor("cbt16", [258, K_CODES], BF16, kind="ExternalInput")
    cbaug = nc.dram_tensor("cbaug", [K_CODES, CAUG], F32, kind="ExternalInput")
    cbpair = nc.dram_tensor("cbpair", [K_CODES // 2, 2 * CAUG], F32, kind="ExternalInput")
    pos2 = nc.dram_tensor("pos2", [TOKS, D_MODEL], BF16, kind="ExternalInput")
    wqk = nc.dram_tensor("wqk", [N_BLOCKS, D_MODEL, 512], BF16, kind="ExternalInput")
    bqk = nc.dram_tensor("bqk", [N_BLOCKS, 512, 1], F32, kind="ExternalInput")
    wv = nc.dram_tensor("wv", [N_BLOCKS, D_MODEL, D_MODEL], BF16, kind="ExternalInput")
    wo = nc.dram_tensor("wo", [N_BLOCKS, D_MODEL, D_MODEL], BF16, kind="ExternalInput")
    bo16 = nc.dram_tensor("bo16", [N_BLOCKS, 1, D_MODEL], BF16, kind="ExternalInput")
    w1 = nc.dram_tensor("w1", [N_BLOCKS, D_MODEL, DFF], BF16, kind="ExternalInput")
    b1 = nc.dram_tensor("b1", [N_BLOCKS, DFF, 1], F32, kind="ExternalInput")
    w2 = nc.dram_tensor("w2", [N_BLOCKS, DFF, D_MODEL], BF16, kind="ExternalInput")
    b216 = nc.dram_tensor("b216", [N_BLOCKS, 1, D_MODEL], BF16, kind="ExternalInput")
    wout = nc.dram_tensor("wout", [D_MODEL, D_PATCH], BF16, kind="ExternalInput")
    bout16 = nc.dram_tensor("bout16", [1, D_PATCH], BF16, kind="ExternalInput")
    logits = nc.dram_tensor("logits", [TOKS, D_PATCH], F32, kind="ExternalOutput")

    with tile.TileContext(nc) as tc:
      for rep in range(reps):
        with (
            tc.tile_pool(name=f"resident{rep}", bufs=1) as res,
            tc.tile_pool(name=f"smalls{rep}", bufs=4) as sm,
        ):
            # residual stream x: [128, 16 tiles x 256] fp32
            xall = res.tile([128, NT * D_MODEL], F32)
            ones16 = res.tile([1, 128], BF16)
            nc.vector.memset(ones16[:], 1.0)
            mln16 = res.tile([128, 1], F32)
            nc.vector.memset(mln16[:], -2.7725887)  # -ln(16): probs scale 1/16
            ident16 = res.tile([128, 128], BF16)
            ident32 = res.tile([128, 128], F32)
            from concourse import masks as _masks
            _masks.make_identity(nc, ident16[:])
            _masks.make_identity(nc, ident32[:])
            # block-0 LN1 output + transpose, built during the VQ DVE drain
            h16_0 = res.tile([128, NT * 256], BF16)
            hT_0 = [res.tile([128, TOKS], BF16, name=f"hT0_{rep}_{d}") for d in range(2)]
            # V augmented with ones cols: per tile t, head h: 64 cols [v(32)|1(32)]
            vaug = res.tile([128, NT * 512], BF16)
            nc.vector.memset(vaug[:], 1.0)
            # block-0 weights + qkT (loaded before VQ so the interleaved
            # qkv-half0 can run during the VQ tail)
            if do_blocks:
                wqk_sb0 = [res.tile([128, 512], BF16, name=f"wqk0_{rep}_{d}") for d in range(2)]
                wv_sb0 = [res.tile([128, 256], BF16, name=f"wv0_{rep}_{d}") for d in range(2)]
                bqkc0 = res.tile([128, 4], F32, name=f"bqkc0_{rep}")
                for d in range(2):
                    nc.sync.dma_start(wqk_sb0[d][:], wqk[0, d * 128:(d + 1) * 128, :])
                    nc.sync.dma_start(wv_sb0[d][:], wv[0, d * 128:(d + 1) * 128, :])
                nc.sync.dma_start(
                    bqkc0[:].rearrange("p (m o) -> p m o", o=1),
                    bqk[0].rearrange("(m p) o -> p m o", p=128))
                qkT_0 = [res.tile([128, TOKS], BF16, name=f"qkT0_{rep}_{m}") for m in range(4)]

            # ================= Phase 1: VQ =================
            if not do_vq:
                posb0 = res.tile([128, NT * D_MODEL], F32)
                nc.sync.dma_start(
                    posb0[:].rearrange("p (t d) -> p t d", d=D_MODEL),
                    pos2[:].rearrange("(t p) d -> p t d", p=128))
                for t in range(NT):
                    nc.vector.tensor_copy(xall[:, t * 256:(t + 1) * 256],
                                          posb0[:, t * 256:(t + 1) * 256])
            if do_vq:
              with (
                tc.tile_pool(name=f"vq_sb{rep}", bufs=1) as vqs,
                tc.tile_pool(name=f"vq_sc{rep}", bufs=2) as vsc,
                tc.tile_pool(name=f"vq_big{rep}", bufs=1) as vbg,
                tc.tile_pool(name=f"vq_sm{rep}", bufs=2) as vsm,
                tc.tile_pool(name=f"vq_ps{rep}", bufs=2, space="PSUM") as vqp,
              ):
                cb0 = vqs.tile([128, K_CODES], BF16)
                cb1 = vqs.tile([128, K_CODES], BF16)
                cb2 = vqs.tile([2, K_CODES], BF16)
                # column-chunked loads so the first score matmuls start after
                # ~1/4 of the codebook transfer instead of all of it
                for cc in range(4):
                    cs_ = slice(cc * 2048, (cc + 1) * 2048)
                    nc.sync.dma_start(cb0[:, cs_], cbt16[0:128, cs_])
                    nc.sync.dma_start(cb1[:, cs_], cbt16[128:256, cs_])
                nc.sync.dma_start(cb2[:], cbt16[256:258, :])
                zt0 = vqs.tile([128, TOKS], BF16)
                zt1 = vqs.tile([128, TOKS], BF16)
                zt2 = vqs.tile([2, TOKS], BF16)
                nc.sync.dma_start(zt0[:], zet16[0:128, :])
                nc.sync.dma_start(zt1[:], zet16[128:256, :])
                nc.sync.dma_start(zt2[:], zet16[256:258, :])
                posb = vqs.tile([128, NT * D_MODEL], BF16)
                nc.sync.dma_start(
                    posb[:].rearrange("p (t d) -> p t d", d=D_MODEL),
                    pos2[:].rearrange("(t p) d -> p t d", p=128))

                for t in range(NT):
                    tsl = slice(t * 128, t * 128 + 128)
                    zea = vsm.tile([128, CAUG], F32, tag="zea", name=f"zea_{rep}_{t}")
                    nc.sync.dma_start(
                        zea[:], zeaug[t * 128:(t + 1) * 128, :])
                    sc16 = vsc.tile([128, K_CODES], F16, tag="sc16")
                    for qtr in range(8):
                        ps = vqp.tile([128, 1024], F32, tag="vq", name=f"vps{rep}_{t}_{qtr}")
                        for ch in range(2):
                            c0 = qtr * 1024 + ch * 512
                            o = ps[:, ch * 512:ch * 512 + 512]
                            nc.tensor.matmul(o, zt0[:, tsl], cb0[:, c0:c0 + 512],
                                             start=True, stop=False)
                            nc.tensor.matmul(o, zt1[:, tsl], cb1[:, c0:c0 + 512],
                                             start=False, stop=False)
                            nc.tensor.matmul(o, zt2[:, tsl], cb2[:, c0:c0 + 512],
                                             start=False, stop=True)
                        nc.scalar.copy(sc16[:, qtr * 1024:qtr * 1024 + 1024], ps[:])
                    # depth-1 max tree: reduce the two 4096-halves (f16, 2x DVE
                    # mode), then max8/max_index scan only 4096 positions. Any
                    # f16-top-4 code's position lands in the top-4 positions of
                    # the reduced array, so rescoring top-4 positions x both
                    # halves (8 exact rescores) covers the true argmin.
                    red = vbg.tile([128, 4096], F16, tag="red")
                    nc.vector.tensor_tensor(out=red[:], in0=sc16[:, 0:4096],
                                            in1=sc16[:, 4096:8192], op=ALU.max)
                    m8 = vsm.tile([128, 8], F16, tag="m8")
                    i8 = vsm.tile([128, 8], U32, tag="i8")
                    nc.vector.max(out=m8[:], in_=red[:])
                    nc.vector.max_index(out=i8[:], in_max=m8[:], in_values=red[:])
                    i4f = vsm.tile([128, 4], F32, tag="i4f")
                    nc.gpsimd.tensor_copy(i4f[:], i8[:, :4])  # u32 -> f32
                    c8f = vsm.tile([128, 8], F32, tag="c8f")  # candidate codes
                    nc.gpsimd.tensor_copy(c8f[:, :4], i4f[:])
                    nc.gpsimd.tensor_scalar(c8f[:, 4:8], i4f[:], 4096.0, None,
                                            op0=ALU.add)
                    c4u = vsm.tile([128, 4], U32, tag="c4u")
                    nc.gpsimd.tensor_copy(c4u[:], i4f[:])  # f32 -> u32 (pair idx)
                    # exact rescore: 4 paired-row gathers (rows p and p+4096
                    # stored adjacently in cbpair), 8 affine rescores
                    cs = vsm.tile([128, NCAND], F32, tag="cs")
                    for k in range(4):
                        gat = vsm.tile([128, 2 * CAUG], F32, tag="gat",
                                       name=f"gat_{rep}_{t}_{k}")
                        nc.gpsimd.indirect_dma_start(
                            out=gat[:], out_offset=None, in_=cbpair[:],
                            in_offset=bass.IndirectOffsetOnAxis(ap=c4u[:, k:k + 1], axis=0))
                        for sub in range(2):
                            scr = vsm.tile([128, CAUG], BF16, tag="scr",
                                           name=f"scr_{rep}_{t}_{k}_{sub}")
                            nc.vector.affine_mul_reduce(
                                out=scr[:], accum_out=cs[:, k + 4 * sub:k + 4 * sub + 1],
                                in0=zea[:],
                                in1=gat[:, sub * CAUG:(sub + 1) * CAUG],
                                scale=1.0, bias=0.0)
                    # pick best (ties -> smallest code index, matching argmin)
                    best = vsm.tile([128, 1], F32, tag="best")
                    nc.vector.reduce_max(best[:], cs[:], axis=mybir.AxisListType.X)
                    eq = vsm.tile([128, NCAND], F32, tag="eq")
                    nc.gpsimd.tensor_scalar(eq[:], cs[:], best[:, :1], None, op0=ALU.is_ge)
                    msk = vsm.tile([128, NCAND], F32, tag="msk")
                    nc.gpsimd.tensor_scalar(msk[:], eq[:], -1e9, 1e9,
                                            op0=ALU.mult, op1=ALU.add)
                    nc.gpsimd.tensor_tensor(out=msk[:], in0=msk[:], in1=c8f[:, :NCAND], op=ALU.add)
                    bidxf = vsm.tile([128, 1], F32, tag="bidxf")
                    nc.vector.tensor_reduce(bidxf[:], msk[:], axis=mybir.AxisListType.X,
                                            op=ALU.min)
                    bidx = vsm.tile([128, 1], U32, tag="bidx")
                    nc.gpsimd.tensor_copy(bidx[:], bidxf[:])  # f32 -> u32
                    zq = vsm.tile([128, CAUG], F32, tag="zq")
                    nc.gpsimd.indirect_dma_start(
                        out=zq[:], out_offset=None, in_=cbaug[:],
                        in_offset=bass.IndirectOffsetOnAxis(ap=bidx[:, :1], axis=0))
                    # x = zq + pos (both SBUF -> Pool is legal and idle here)
                    nc.gpsimd.tensor_tensor(
                        out=xall[:, t * 256:(t + 1) * 256], in0=zq[:, :256],
                        in1=posb[:, t * 256:(t + 1) * 256], op=ALU.add)
                    if do_blocks and t in (7, 15):
                        # block-0 LN1 + transposes for this batch half run on
                        # ACT/Pool/PE while DVE drains the remaining VQ tiles
                        bh = t // 8
                        st0 = _ln_stats(nc, sm, xall, "pre", "ln1", bh)
                        _ln_apply(nc, xall, h16_0, st0, bh, pool_only=True)
                        _transpose_to(nc, tc, ident16, hT_0, h16_0,
                                      f"hT0_{rep}_{bh}", tt0=bh * 8, tt1=bh * 8 + 8,
                                      act_only=True)
                    if do_blocks and t == 7:
                        # block-0 qkv (batch half 0) + v quarters 0,1 overlap
                        # the VQ DVE drain of tiles 8-15 (qp: 4 PSUM banks)
                        with tc.tile_pool(name=f"qkv0_ps_{rep}", bufs=2,
                                          space="PSUM") as qp0:
                            _qkv_half(nc, qp0, qkT_0, hT_0, wqk_sb0, bqkc0,
                                      vaug, wv_sb0, f"{rep}_pre", 0, act_only=True)

            # ================= Phase 2: transformer blocks =================
            pending = {}
            with (
                tc.tile_pool(name=f"blk_w{rep}", bufs=2) as bw,
                tc.tile_pool(name=f"blk_act{rep}", bufs=1) as ba,
                tc.tile_pool(name=f"blk_probs{rep}", bufs=8) as bpr,
                tc.tile_pool(name=f"blk_sm{rep}", bufs=4) as bsm,
                tc.tile_pool(name=f"blk_nrm{rep}", bufs=2) as bnr,
            ):
                for blk0 in (range(N_BLOCKS) if do_blocks else []):
                    blk = blk0  # numeric index for DRAM slicing
                    nm = f"{rep}_{blk0}"
                    # ---- load weights for this block
                    wqk_sb = [bw.tile([128, 512], BF16, tag=f"wqk{d}", name=f"wqk_{nm}_{d}") for d in range(2)]
                    wv_sb = [bw.tile([128, 256], BF16, tag=f"wv{d}", name=f"wv_{nm}_{d}") for d in range(2)]
                    wo_sb = [bw.tile([128, 256], BF16, tag=f"wo{d}", name=f"wo_{nm}_{d}") for d in range(2)]
                    w1_sb = [bw.tile([128, DFF], BF16, tag=f"w1{d}", name=f"w1_{nm}_{d}") for d in range(2)]
                    w2c = bw.tile([128, 8 * 256], BF16, tag="w2c", name=f"w2c_{nm}")
                    w2_sb = [w2c[:, d * 256:(d + 1) * 256] for d in range(8)]
                    skip_qkvw = blk == 0 and do_vq
                    for d in range(2):
                        if not skip_qkvw:
                            nc.sync.dma_start(wqk_sb[d][:], wqk[blk, d * 128:(d + 1) * 128, :])
                            nc.sync.dma_start(wv_sb[d][:], wv[blk, d * 128:(d + 1) * 128, :])
                        nc.sync.dma_start(wo_sb[d][:], wo[blk, d * 128:(d + 1) * 128, :])
                        nc.sync.dma_start(w1_sb[d][:], w1[blk, d * 128:(d + 1) * 128, :])
                    nc.sync.dma_start(
                        w2c[:].rearrange("p (d m) -> p d m", m=256),
                        w2[blk].rearrange("(d p) m -> p d m", p=128))
                    # consolidated bias loads: one DMA each instead of 4+8 tiny ones
                    bqkc = bw.tile([128, 4], F32, tag="bqkc", name=f"bqkc_{nm}")
                    if not skip_qkvw:
                        nc.sync.dma_start(
                            bqkc[:].rearrange("p (m o) -> p m o", o=1),
                            bqk[blk].rearrange("(m p) o -> p m o", p=128))
                    b1c = bw.tile([128, 8], F32, tag="b1c", name=f"b1c_{nm}")
                    nc.sync.dma_start(
                        b1c[:].rearrange("p (m o) -> p m o", o=1),
                        b1[blk].rearrange("(m p) o -> p m o", p=128))
                    bo_sb = bw.tile([1, 256], BF16, tag="bo", name=f"bo_{nm}")
                    b2_sb = bw.tile([1, 256], BF16, tag="b2", name=f"b2_{nm}")
                    nc.sync.dma_start(bo_sb[:], bo16[blk])
                    nc.sync.dma_start(b2_sb[:], b216[blk])

                    # ---- LN1 -> h16 (bf16) ; gamma/beta folded into weights
                    if blk == 0 and do_vq:
                        hT = hT_0
                    elif blk in pending:
                        h16, hT = pending.pop(blk)
                        st1 = _ln_stats(nc, bsm, xall, nm, "ln1", 1)
                        _ln_apply(nc, xall, h16, st1, 1)
                        _transpose_to(nc, tc, ident16, hT, h16, f"hT{nm}b",
                                      tt0=8, tt1=16)
                    else:
                        h16 = ba.tile([128, NT * 256], BF16, tag="h16", name=f"h16_{nm}")
                        _layernorm(nc, tc, bsm, xall, h16, nm, "ln1")
                        # ---- hT via PE transposes
                        hT = [ba.tile([128, TOKS], BF16, tag=f"hT{d}", name=f"hT_{nm}_{d}") for d in range(2)]
                        _transpose_to(nc, tc, ident16, hT, h16, f"hT{nm}")

                    # ---- qkT (feature-major) and vaug (token-major, [v|1] cols)
                    if blk == 0 and do_vq:
                        qkT = qkT_0
                        wqk_u, wv_u, bqk_u = wqk_sb0, wv_sb0, bqkc0
                    else:
                        qkT = [ba.tile([128, TOKS], BF16, tag=f"qkT{m}", name=f"qkT_{nm}_{m}") for m in range(4)]
                        wqk_u, wv_u, bqk_u = wqk_sb, wv_sb, bqkc
                    with tc.tile_pool(name=f"qkv_ps_{rep}_{blk}", bufs=2, space="PSUM") as qp:
                        for half in range(2):
                            if blk == 0 and do_vq and half == 0:
                                continue  # emitted inside the VQ scope
                            _qkv_half(nc, qp, qkT, hT, wqk_u, bqk_u,
                                      vaug, wv_u, nm, half)

                    # ---- attention (batch-major) fused with S3 + LN2 stats:
                    # after batch b's two head-groups finish, its S3 quarters
                    # and LN2 stats run while batch b+1's attention proceeds.
                    oT16 = [ba.tile([128, SEQLEN], BF16, tag=f"oT{i}", name=f"oT_{nm}_{i}")
                            for i in range(4)]  # (b, g) -> [4heads*32, 1024 q]
                    h2 = ba.tile([128, NT * 256], BF16, tag="h16", name=f"h2_{nm}")
                    ln2st = []

                    def _emit_s3(b, s3pool):
                        for g4 in (2 * b, 2 * b + 1):
                            po = s3pool.tile([128, 1024], F32, tag="po", name=f"po_{nm}_{g4}")
                            for j in range(4):
                                t = g4 * 4 + j
                                q = (t % TPB) * 128
                                o = po[:, j * 256:(j + 1) * 256]
                                for g in range(2):
                                    nc.tensor.matmul(o, oT16[b * 2 + g][:, q:q + 128],
                                                     wo_sb[g][:], start=(g == 0), stop=False)
                                nc.tensor.matmul(o, ones16[:, :128], bo_sb[:],
                                                 start=False, stop=True)
                            w = slice(g4 * 1024, (g4 + 1) * 1024)
                            nc.vector.tensor_tensor(
                                out=xall[:, w], in0=xall[:, w], in1=po[:], op=ALU.add)

                    if True:
                        with (
                            tc.tile_pool(name=f"attn_s1_{nm}", bufs=4, space="PSUM") as s1p,
                            tc.tile_pool(name=f"attn_acc_{nm}", bufs=1, space="PSUM") as accp,
                            tc.tile_pool(name=f"s3_ps_{rep}_{blk}", bufs=1, space="PSUM") as s3p,
                        ):
                            for b in range(2):
                                for g in range(2):
                                    _attn_group(nc, tc, bpr, bnr, qkT, vaug, mln16,
                                                oT16[b * 2 + g], nm, b, g, s1p, accp)
                                _emit_s3(b, s3p)
                                ln2st.append(_ln_stats(nc, bsm, xall, nm, "ln2", b))

                    # ---- LN2 apply -> h2 -> h2T
                    for b in range(2):
                        _ln_apply(nc, xall, h2, ln2st[b], b)
                    h2T = [ba.tile([128, TOKS], BF16, tag=f"hT{d}", name=f"h2T_{nm}_{d}") for d in range(2)]
                    _transpose_to(nc, tc, ident16, h2T, h2, f"h2T{nm}")

                    # ---- FFN: gT = gelu(W1T h2T + b1) ; x += gT.T @ W2 + b2
                    # hh-outer so FFN2's first token chunks unblock after 8 gelus
                    gT = [ba.tile([128, TOKS], BF16, tag=f"gT{p}", name=f"gT_{nm}_{p}") for p in range(8)]
                    with tc.tile_pool(name=f"ffn1_ps_{rep}_{blk}", bufs=3, space="PSUM") as f1p:
                        for hh in range(2):
                            for p in range(8):
                                pu = f1p.tile([128, 1024], F32, tag="pu", name=f"pu_{nm}_{p}_{hh}")
                                for ch in range(2):
                                    c0 = hh * 1024 + ch * 512
                                    o = pu[:, ch * 512:ch * 512 + 512]
                                    nc.tensor.matmul(o, w1_sb[0][:, p * 128:(p + 1) * 128],
                                                     h2T[0][:, c0:c0 + 512], start=True, stop=False)
                                    nc.tensor.matmul(o, w1_sb[1][:, p * 128:(p + 1) * 128],
                                                     h2T[1][:, c0:c0 + 512], start=False, stop=True)
                                nc.scalar.activation(gT[p][:, hh * 1024:(hh + 1) * 1024], pu[:],
                                                     AF.Gelu_apprx_tanh, bias=b1c[:, p:p + 1])
                    with tc.tile_pool(name=f"ffn2_ps_{rep}_{blk}", bufs=3, space="PSUM") as f2p:
                        for g4 in range(4):
                            pf = f2p.tile([128, 1024], F32, tag="pf", name=f"pf_{nm}_{g4}")
                            for j in range(4):
                                t = g4 * 4 + j
                                tsl = slice(t * 128, t * 128 + 128)
                                o = pf[:, j * 256:(j + 1) * 256]
                                for p in range(8):
                                    nc.tensor.matmul(o, gT[p][:, tsl],
                                                     w2c[:, p * 256:(p + 1) * 256],
                                                     start=(p == 0), stop=False)
                                nc.tensor.matmul(o, ones16[:, :128], b2_sb[:],
                                                 start=False, stop=True)
                            w = slice(g4 * 1024, (g4 + 1) * 1024)
                            nc.vector.tensor_tensor(
                                out=xall[:, w], in0=xall[:, w], in1=pf[:], op=ALU.add)


                # ================= Phase 3: output projection =================
                wout_sb = [bw.tile([128, D_PATCH], BF16, tag=f"wout{d}", name=f"wout_{rep}_{d}") for d in range(2)]
                for d in range(2):
                    nc.sync.dma_start(wout_sb[d][:], wout[d * 128:(d + 1) * 128, :])
                bout_sb = bw.tile([1, D_PATCH], BF16, tag="bout", name=f"bout_{rep}")
                nc.sync.dma_start(bout_sb[:], bout16[:])
                # transpose fp32 xall directly (2 cy/row), psum -> bf16 xT
                xT = [ba.tile([128, TOKS], BF16, tag=f"hT{d}", name=f"xT_{d}") for d in range(2)]
                _transpose_fp32(nc, tc, ident32, xT, xall, f"xTo_{rep}f", 0, NT)
                with (
                    tc.tile_pool(name=f"out_ps{rep}", bufs=4, space="PSUM") as op,
                    tc.tile_pool(name=f"out_st{rep}", bufs=2) as ost,
                ):
                    # stage 2 tiles per DMA: single-DMA issue overhead (~2us on
                    # the SP queue) otherwise serializes the whole tail
                    for tq in range(8):
                        ol4 = ost.tile([128, 2 * D_PATCH], F32, tag="ol4",
                                       name=f"ol4_{rep}_{tq}")
                        for j in range(2):
                            t = tq * 2 + j
                            tsl = slice(t * 128, t * 128 + 128)
                            pl = op.tile([128, 1024], F32, tag="pl", name=f"pl_{rep}_{t}")
                            for ch, w in ((0, 512), (1, 256)):
                                o = pl[:, ch * 512:ch * 512 + w]
                                nc.tensor.matmul(o, xT[0][:, tsl], wout_sb[0][:, ch * 512:ch * 512 + w],
                                                 start=True, stop=False)
                                nc.tensor.matmul(o, xT[1][:, tsl], wout_sb[1][:, ch * 512:ch * 512 + w],
                                                 start=False, stop=False)
                                nc.tensor.matmul(o, ones16[:, :128], bout_sb[:, ch * 512:ch * 512 + w],
                                                 start=False, stop=True)
                            dst = ol4[:, j * D_PATCH:(j + 1) * D_PATCH]
                            if t % 2 == 0:
                                nc.scalar.copy(dst, pl[:, :768])
                            else:
                                nc.vector.tensor_copy(dst, pl[:, :768])
                        nc.sync.dma_start(
                            logits[tq * 256:(tq + 1) * 256, :].rearrange(
                                "(t p) d -> p t d", p=128),
                            ol4[:].rearrange("p (t d) -> p t d", d=D_PATCH))

    nc.compile()
    return nc


def _qkv_half(nc, qp, qkT, hT, wqk_sb, bqkc, vaug, wv_sb, nm, half, act_only=False):
    """q/k projections for one batch half (512-wide PSUM chunks, 4 banks) and
    the V quarters of that half written into vaug's [v|1] layout."""
    c0h = half * 1024
    for m in range(4):
        for sub in range(2):
            pq = qp.tile([128, 512], F32, tag="pq", name=f"pq_{nm}_{m}_{half}_{sub}")
            c0 = c0h + sub * 512
            nc.tensor.matmul(pq[:], wqk_sb[0][:, m * 128:(m + 1) * 128],
                             hT[0][:, c0:c0 + 512], start=True, stop=False)
            nc.tensor.matmul(pq[:], wqk_sb[1][:, m * 128:(m + 1) * 128],
                             hT[1][:, c0:c0 + 512], start=False, stop=True)
            if (m + sub) % 2 == 0 and not act_only:
                nc.vector.tensor_scalar(qkT[m][:, c0:c0 + 512], pq[:],
                                        bqkc[:, m:m + 1], None, op0=ALU.add)
            else:
                nc.scalar.activation(qkT[m][:, c0:c0 + 512], pq[:],
                                     AF.Identity, bias=bqkc[:, m:m + 1])
    for g4 in (2 * half, 2 * half + 1):
        for s2 in range(2):
            pv = qp.tile([128, 512], F32, tag="pv", name=f"pv_{nm}_{g4}_{s2}")
            for j in range(2):
                t = g4 * 4 + s2 * 2 + j
                tsl = slice(t * 128, t * 128 + 128)
                o = pv[:, j * 256:(j + 1) * 256]
                nc.tensor.matmul(o, hT[0][:, tsl], wv_sb[0][:], start=True, stop=False)
                nc.tensor.matmul(o, hT[1][:, tsl], wv_sb[1][:], start=False, stop=True)
            # strided copy into vaug dims columns ([v|1] layout)
            base = g4 * 2048 + s2 * 1024
            vdst = vaug[:, base:base + 1024].rearrange(
                "p (t h e) -> p t h e", h=8, e=64)[:, :, :, 0:32]
            vsrc = pv[:].rearrange("p (t h e) -> p t h e", h=8, e=32)
            if (g4 + s2) % 2 == 0 and not act_only:
                nc.vector.tensor_copy(vdst, vsrc)
            else:
                nc.scalar.copy(vdst, vsrc)


def _transpose_fp32(nc, tc, ident32, dstT, xall, tag, tt0, tt1, bufs=4):
    """bf16 transposes of the fp32 residual (2 cy/row on PE)."""
    with tc.tile_pool(name=f"tp_ps_{tag}", bufs=bufs, space="PSUM") as tpp:
        for tt in range(tt0, tt1, 4):
            for d in range(2):
                pt = tpp.tile([128, 512], F32, tag="tp", name=f"tp_{tag}_{tt}_{d}")
                for j in range(4):
                    t = tt + j
                    nc.tensor.transpose(
                        pt[:, j * 128:(j + 1) * 128],
                        xall[:, t * 256 + d * 128: t * 256 + (d + 1) * 128],
                        ident32[:])
                if (tt // 4 + d) % 2 == 0:
                    nc.vector.tensor_copy(dstT[d][:, tt * 128:(tt + 4) * 128], pt[:])
                else:
                    nc.scalar.copy(dstT[d][:, tt * 128:(tt + 4) * 128], pt[:])


def _transpose_to(nc, tc, ident16, dstT, src, tag, tt0=0, tt1=NT, act_only=False, bufs=4):
    """dstT[d][:, t*128:(t+1)*128] = transpose(src[:, t*256+d*128 : ...]) via PE.
    4 transposes share one PSUM tile; single batched copy, DVE/ACT alternated."""
    with tc.tile_pool(name=f"tp_ps_{tag}", bufs=bufs, space="PSUM") as tpp:
        for tt in range(tt0, tt1, 4):
            for d in range(2):
                pt = tpp.tile([128, 512], BF16, tag="tp", name=f"tp_{tag}_{tt}_{d}")
                for j in range(4):
                    t = tt + j
                    nc.tensor.transpose(
                        pt[:, j * 128:(j + 1) * 128],
                        src[:, t * 256 + d * 128: t * 256 + (d + 1) * 128],
                        ident16[:])
                if (tt // 4 + d) % 2 == 0 and not act_only:
                    nc.vector.tensor_copy(dstT[d][:, tt * 128:(tt + 4) * 128], pt[:])
                else:
                    nc.scalar.copy(dstT[d][:, tt * 128:(tt + 4) * 128], pt[:])


def _ln_stats(nc, bsm, xall, blk, tag, bh):
    """bn stats + rstd for batch-half bh (tiles bh*8..bh*8+8).
    rstd computed as exp(-0.5 * ln(var + eps))."""
    NH = NT // 2
    t0 = bh * NH
    epsc = bsm.tile([128, 1], F32, tag="lneps", name=f"{tag}eps_{blk}_{bh}")
    nc.vector.memset(epsc[:], LN_EPS)
    meancol = bsm.tile([128, NH], F32, tag="lnmean", name=f"{tag}mean_{blk}_{bh}")
    varcol = bsm.tile([128, NH], F32, tag="lnvar", name=f"{tag}var_{blk}_{bh}")
    rstdcol = bsm.tile([128, NH], F32, tag="lnrstd", name=f"{tag}rstd_{blk}_{bh}")
    lncol = bsm.tile([128, NH], F32, tag="lnln", name=f"{tag}ln_{blk}_{bh}")
    for i in range(NH):
        t = t0 + i
        stats = bsm.tile([128, 6], F32, tag="lnstats", name=f"{tag}stats_{blk}_{t}")
        mv = bsm.tile([128, 2], F32, tag="lnmv", name=f"{tag}mv_{blk}_{t}")
        nc.vector.bn_stats(stats[:], xall[:, t * 256:(t + 1) * 256])
        nc.vector.bn_aggr(mv[:], stats[:])
        nc.vector.tensor_copy(meancol[:, i:i + 1], mv[:, 0:1])
        nc.vector.tensor_copy(varcol[:, i:i + 1], mv[:, 1:2])
    nc.scalar.activation(lncol[:], varcol[:], AF.Ln, bias=epsc[:, :1])
    nc.scalar.activation(rstdcol[:], lncol[:], AF.Exp, scale=-0.5)
    return meancol, rstdcol


def _ln_apply(nc, xall, h16, st, bh, pool_only=False):
    """h16 tiles of batch-half bh = (x - mean) * rstd, Pool/DVE alternated."""
    meancol, rstdcol = st
    NH = NT // 2
    t0 = bh * NH
    for i in range(NH):
        t = t0 + i
        eng = nc.gpsimd if (pool_only or i % 2 == 0) else nc.vector
        eng.tensor_scalar(
            h16[:, t * 256:(t + 1) * 256], xall[:, t * 256:(t + 1) * 256],
            meancol[:, i:i + 1], rstdcol[:, i:i + 1],
            op0=ALU.subtract, op1=ALU.mult)


def _layernorm(nc, tc, bsm, xall, h16, blk, tag):
    for bh in range(2):
        st = _ln_stats(nc, bsm, xall, blk, tag, bh)
        _ln_apply(nc, xall, h16, st, bh)



def _attn_group(nc, tc, bpr, bnr, qkT, vaug, mln16, oT_out, blk, b, g, s1p, accp, exp_pat="AADAD"):
    """One (batch, 4-head-group): scoresT QK matmuls -> exp -> fused PV+rowsum
    matmuls ([v|1] 64-wide lhsT -> psum bands [d(32), s(32)] per head) ->
    normalize via base-shifted reciprocal + same-base mult.
    qc-outer with [128, 512] accumulators so attention holds only 6 PSUM banks.
    oT_out: [128 (4h x 32 dims), 1024 q] bf16."""
    name = f"{blk}_{b}_{g}"
    qt, kt = qkT[g], qkT[2 + g]
    tok0 = b * SEQLEN
    ci = 0
    for qc in range(2):
        qs = slice(qc * 512, qc * 512 + 512)
        # otp[half]: [d_h0(0:32), s_h0(32:64), d_h1(64:96), s_h1(96:128)] x 512 q
        otp = [accp.tile([128, 512], F32, tag=f"otp{half}",
                         name=f"otp_{name}_{qc}_{half}") for half in range(2)]
        for kk in range(8):
            vbase = (b * TPB + kk) * 512 + g * 256
            for half in range(2):
                for hh in range(2):
                    h = half * 2 + hh
                    # fine-grained per-head [128, 512] chunk: QK -> exp -> PV.
                    ps = s1p.tile([128, 512], F32, tag="s1",
                                  name=f"s1_{name}_{qc}_{kk}_{half}_{hh}")
                    nc.tensor.matmul(
                        ps[:],
                        kt[32 * h:32 * h + 32, tok0 + kk * 128: tok0 + (kk + 1) * 128],
                        qt[32 * h:32 * h + 32, tok0 + qc * 512: tok0 + (qc + 1) * 512],
                        start=True, stop=True, tile_position=(32 * h, 0))
                    pr = bpr.tile([128, 512], BF16, tag="probs",
                                  name=f"pr_{name}_{qc}_{kk}_{half}_{hh}")
                    # probs scaled by 1/16 (exp(s)-ln16) so f16 sums stay in
                    # range; the d/s ratio is scale-invariant.
                    eng = exp_pat[ci % len(exp_pat)]
                    ci += 1
                    if eng == "A":
                        nc.scalar.activation(pr[:], ps[:], AF.Exp, bias=mln16[:, :1])
                    else:
                        with nc.allow_low_precision(reason="approx softmax exp"):
                            nc.vector.tensor_scalar(
                                pr.bitcast(mybir.dt.uint16)[:], ps[:],
                                184.66496, 15738.5, op0=ALU.mult, op1=ALU.add)
                    # fused PV+rowsum: lhsT = [v_h(32) | ones(32)]
                    nc.tensor.matmul(otp[half][hh * 64:(hh + 1) * 64, :],
                                     vaug[:, vbase + h * 64: vbase + (h + 1) * 64],
                                     pr[:], start=(kk == 0), stop=(kk == 7),
                                     tile_position=(0, 64 * hh))
        # normalize this qc window: oT[h band] = d_h * (1 / s_h)
        for half in range(2):
            sb_acc = bnr.tile([128, 512], F16, tag=f"sbac{half}",
                              name=f"sbac_{name}_{qc}_{half}")
            nc.scalar.copy(sb_acc[:], otp[half][:])
            recS = bnr.tile([128, 512], F16, tag=f"recS{half}",
                            name=f"recS_{name}_{qc}_{half}")
            with nc.allow_low_precision(reason="softmax recip consumed by bf16 probs"):
                nc.vector.reciprocal(recS[0:32, :], sb_acc[32:64, :])
                nc.vector.reciprocal(recS[64:96, :], sb_acc[96:128, :])
            for hh in range(2):
                h = half * 2 + hh
                nc.vector.tensor_tensor(
                    out=oT_out[32 * h:32 * h + 32, qs], in0=sb_acc[64 * hh:64 * hh + 32, :],
                    in1=recS[64 * hh:64 * hh + 32, :], op=ALU.mult)



def _prep_inputs(inputs):
    """Host-side prep: shard, transpose, fold LN scales, cast."""
    ze = np.asarray(inputs["ze"], np.float32)
    cb = np.asarray(inputs["codebook"], np.float32)
    pos = np.asarray(inputs["pos_emb"], np.float32)
    n2 = (cb * cb).sum(-1)
    n2h = n2.astype(ml_dtypes.bfloat16).astype(np.float32)
    n2l = (n2 - n2h).astype(ml_dtypes.bfloat16).astype(np.float32)

    cbt = np.empty((258, K_CODES), np.float32)
    cbt[:256] = cb.T
    cbt[256] = -n2h
    cbt[257] = -n2l
    cbt16 = _bf16(cbt)
    cbaug = np.zeros((K_CODES, CAUG), np.float32)
    cbaug[:, :256] = cb
    cbaug[:, 256] = n2
    cbpair = np.concatenate([cbaug[:K_CODES // 2], cbaug[K_CODES // 2:]], axis=1)
    pos2 = _bf16(np.concatenate([pos] * B_PER_CORE, axis=0))

    shared = {"cbt16": cbt16, "cbaug": cbaug, "cbpair": cbpair, "pos2": pos2}

    sq = 1.0 / np.sqrt(np.float32(D_MODEL // N_HEADS))
    wqk_l, bqk_l, wv_l, wo_l, bo_l = [], [], [], [], []
    w1_l, b1_l, w2_l, b2_l = [], [], [], []
    for i in range(N_BLOCKS):
        Wqkv = np.asarray(inputs["Wqkv"][i], np.float32)
        bqkv = np.asarray(inputs["bqkv"][i], np.float32)
        s1v = np.asarray(inputs["ln1_s"][i], np.float32)
        b1v = np.asarray(inputs["ln1_b"][i], np.float32)
        Wf = s1v[:, None] * Wqkv
        bf = bqkv + b1v @ Wqkv
        Wf[:, :256] *= sq
        bf[:256] *= sq
        wqk_l.append(_bf16(Wf[:, :512]))
        bqk_l.append(bf[:512].astype(np.float32)[:, None])
        wv_l.append(_bf16(Wf[:, 512:]))
        Wo = np.asarray(inputs["Wo"][i], np.float32)
        wo_l.append(_bf16(Wo))
        # v bias folded into output-proj bias: softmax-weighted mean of a
        # constant is the constant, so attn(v + bv) @ Wo = attn(v) @ Wo + bv @ Wo
        bv = bf[512:]
        bo_l.append(_bf16((np.asarray(inputs["bo"][i], np.float32) + bv @ Wo)[None, :]))
        W1 = np.asarray(inputs["W1"][i], np.float32)
        s2v = np.asarray(inputs["ln2_s"][i], np.float32)
        b2v = np.asarray(inputs["ln2_b"][i], np.float32)
        w1_l.append(_bf16(s2v[:, None] * W1))
        b1_l.append((np.asarray(inputs["b1"][i], np.float32) + b2v @ W1).astype(np.float32)[:, None])
        w2_l.append(_bf16(inputs["W2"][i]))
        b2_l.append(_bf16(np.asarray(inputs["b2"][i], np.float32)[None, :]))
    shared.update({
        "wqk": np.stack(wqk_l), "bqk": np.stack(bqk_l),
        "wv": np.stack(wv_l),
        "wo": np.stack(wo_l), "bo16": np.stack(bo_l),
        "w1": np.stack(w1_l), "b1": np.stack(b1_l),
        "w2": np.stack(w2_l), "b216": np.stack(b2_l),
        "wout": _bf16(inputs["Wout"]),
        "bout16": _bf16(np.asarray(inputs["bout"], np.float32)[None, :]),
    })

    in_maps = []
    for c in range(N_CORES):
        zec = ze[c * B_PER_CORE:(c + 1) * B_PER_CORE].reshape(TOKS, D_MODEL)
        zet = np.empty((258, TOKS), np.float32)
        zet[:256] = (2.0 * zec).T
        zet[256:258] = 1.0
        zeaug = np.zeros((TOKS, CAUG), np.float32)
        zeaug[:, :256] = 2.0 * zec
        zeaug[:, 256] = -1.0
        in_maps.append({**shared, "zet16": _bf16(zet), "zeaug": zeaug})
    return in_maps


def kernel(**inputs) -> np.ndarray:
    if "nc" not in _BUILD_CACHE:
        _BUILD_CACHE["nc"] = build_nc("full", reps=1)
    nc = _BUILD_CACHE["nc"]
    in_maps = _prep_inputs(inputs)
    res = bass_utils.run_bass_kernel_spmd(nc, in_maps, core_ids=list(range(N_CORES)))
    out = np.stack([res.results[c]["logits"] for c in range(N_CORES)])
    return out.reshape(BATCH, SEQLEN, D_PATCH)


# revision 50
# speedup vs baseline: 2.4777x; 2.4777x over previous
"""VQ-VAE decoder (vq_codebook) on 8 TRN2 NeuronCores, batch-sharded.

Pipeline per core (2 batch elements = 2048 tokens):
  1. VQ: scores = 2*ze.c - ||c||^2 via bf16 matmul (n2 folded in as two extra
     bf16 contraction rows), fp16 scores -> DVE max/max_index top-8, exact
     fp32 rescore of top-4 candidates (paired-row gather + affine_mul_reduce),
     gather winning codebook row, add pos_emb.
  2. 4 transformer blocks (pre-LN MHA + pre-LN FFN), bf16 matmuls:
     - LN stats on DVE (bn_stats/bn_aggr), rstd via exp(-0.5*ln(var+eps)),
       LN gamma/beta folded into the following weights host-side.
     - h -> hT via PE transposes; qkvT layout (features on partitions).
     - Attention: scoresT orientation, 4-head row-tiled QK matmuls, ACT exp
       (softmax max-subtraction skipped; scores are O(1)), fused PV+rowsum
       matmuls via V-weights augmented with ones columns ([v|1] 64-wide
       lhsT -> psum bands [dims, sums]), normalize via base-shifted
       reciprocal + mult. V bias folded into Wo bias host-side.
  3. Output projection x @ Wout + bout -> logits fp32 (fp32 transpose of the
     residual, 2-tile staged output DMAs to amortize SP-queue issue cost).

Self-contained: hardcodes all shapes; host preps transposed/augmented/bf16
operands and LN-folded weights; runs SPMD on cores 0-7 and concatenates.
"""

import os
import numpy as np
import ml_dtypes

import concourse.bass as bass
import concourse.bacc as bacc
import concourse.mybir as mybir
from concourse import tile
from concourse import bass_utils

F32 = mybir.dt.float32
F16 = mybir.dt.float16
BF16 = mybir.dt.bfloat16
U32 = mybir.dt.uint32
U16 = mybir.dt.uint16
AF = mybir.ActivationFunctionType
ALU = mybir.AluOpType

D_MODEL, D_PATCH, K_CODES, SEQLEN = 256, 768, 8192, 1024
N_HEADS, N_BLOCKS, DFF = 8, 4, 1024
BATCH, N_CORES = 16, 8
LN_EPS = 1e-5
B_PER_CORE = BATCH // N_CORES            # 2
TOKS = B_PER_CORE * SEQLEN               # 2048
NT = TOKS // 128                         # 16 token tiles
TPB = SEQLEN // 128                      # 8 token tiles per batch element
NCAND = 8                                # exact-rescore candidates (4 pos x 2 halves)
CAUG = 264                               # padded augmented row width (256+1 pad to 8)

_BUILD_CACHE = {}


def _bf16(x):
    return np.asarray(x, np.float32).astype(ml_dtypes.bfloat16)


def build_nc(variant=None, reps=1):
    variant = variant or os.environ.get("KVAR", "full")
    do_vq = variant in ("full", "noblocks")
    do_blocks = variant in ("full", "novq")
    nc = bacc.Bacc("TRN2", target_bir_lowering=False)

    # ---------------- DRAM I/O ----------------
    zet16 = nc.dram_tensor("zet16", [258, TOKS], BF16, kind="ExternalInput")
    zeaug = nc.dram_tensor("zeaug", [TOKS, CAUG], F32, kind="ExternalInput")
    cbt16 = nc.dram_tensor("cbt16", [258, K_CODES], BF16, kind="ExternalInput")
    cbaug = nc.dram_tensor("cbaug", [K_CODES, CAUG], F32, kind="ExternalInput")
    cbpair = nc.dram_tensor("cbpair", [K_CODES // 2, 2 * CAUG], F32, kind="ExternalInput")
    pos2 = nc.dram_tensor("pos2", [TOKS, D_MODEL], BF16, kind="ExternalInput")
    wqk = nc.dram_tensor("wqk", [N_BLOCKS, D_MODEL, 512], BF16, kind="ExternalInput")
    bqk = nc.dram_tensor("bqk", [N_BLOCKS, 512, 1], F32, kind="ExternalInput")
    wv = nc.dram_tensor("wv", [N_BLOCKS, D_MODEL, D_MODEL], BF16, kind="ExternalInput")
    wo = nc.dram_tensor("wo", [N_BLOCKS, D_MODEL, D_MODEL], BF16, kind="ExternalInput")
    bo16 = nc.dram_tensor("bo16", [N_BLOCKS, 1, D_MODEL], BF16, kind="ExternalInput")
    w1 = nc.dram_tensor("w1", [N_BLOCKS, D_MODEL, DFF], BF16, kind="ExternalInput")
    b1 = nc.dram_tensor("b1", [N_BLOCKS, DFF, 1], F32, kind="ExternalInput")
    w2 = nc.dram_tensor("w2", [N_BLOCKS, DFF, D_MODEL], BF16, kind="ExternalInput")
    b216 = nc.dram_tensor("b216", [N_BLOCKS, 1, D_MODEL], BF16, kind="ExternalInput")
    wout = nc.dram_tensor("wout", [D_MODEL, D_PATCH], BF16, kind="ExternalInput")
    bout16 = nc.dram_tensor("bout16", [1, D_PATCH], BF16, kind="ExternalInput")
    logits = nc.dram_tensor("logits", [TOKS, D_PATCH], F32, kind="ExternalOutput")

    with tile.TileContext(nc) as tc:
      for rep in range(reps):
        with (
            tc.tile_pool(name=f"resident{rep}", bufs=1) as res,
            tc.tile_pool(name=f"smalls{rep}", bufs=4) as sm,
        ):
            # residual stream x: [128, 16 tiles x 256] fp32
            xall = res.tile([128, NT * D_MODEL], F32)
            ones16 = res.tile([1, 128], BF16)
            nc.vector.memset(ones16[:], 1.0)
            mln16 = res.tile([128, 1], F32)
            nc.vector.memset(mln16[:], -2.7725887)  # -ln(16): probs scale 1/16
            ident16 = res.tile([128, 128], BF16)
            ident32 = res.tile([128, 128], F32)
            from concourse import masks as _masks
            _masks.make_identity(nc, ident16[:])
            _masks.make_identity(nc, ident32[:])
            # block-0 LN1 output + transpose, built during the VQ DVE drain
            h16_0 = res.tile([128, NT * 256], BF16)
            hT_0 = [res.tile([128, TOKS], BF16, name=f"hT0_{rep}_{d}") for d in range(2)]
            # V augmented with ones cols: per tile t, head h: 64 cols [v(32)|1(32)]
            vaug = res.tile([128, NT * 512], BF16)
            nc.gpsimd.memset(vaug[:], 1.0)
            # block-0 weights + qkT (loaded before VQ so the interleaved
            # qkv-half0 can run during the VQ tail)
            if do_blocks:
                wqk_sb0 = [res.tile([128, 512], BF16, name=f"wqk0_{rep}_{d}") for d in range(2)]
                wv_sb0 = [res.tile([128, 256], BF16, name=f"wv0_{rep}_{d}") for d in range(2)]
                bqkc0 = res.tile([128, 4], F32, name=f"bqkc0_{rep}")
                for d in range(2):
                    nc.sync.dma_start(wqk_sb0[d][:], wqk[0, d * 128:(d + 1) * 128, :])
                    nc.sync.dma_start(wv_sb0[d][:], wv[0, d * 128:(d + 1) * 128, :])
                nc.sync.dma_start(
                    bqkc0[:].rearrange("p (m o) -> p m o", o=1),
                    bqk[0].rearrange("(m p) o -> p m o", p=128))
                qkT_0 = [res.tile([128, TOKS], BF16, name=f"qkT0_{rep}_{m}") for m in range(4)]

            # ================= Phase 1: VQ =================
            if not do_vq:
                posb0 = res.tile([128, NT * D_MODEL], F32)
                nc.sync.dma_start(
                    posb0[:].rearrange("p (t d) -> p t d", d=D_MODEL),
                    pos2[:].rearrange("(t p) d -> p t d", p=128))
                for t in range(NT):
                    nc.vector.tensor_copy(xall[:, t * 256:(t + 1) * 256],
                                          posb0[:, t * 256:(t + 1) * 256])
            if do_vq:
              with (
                tc.tile_pool(name=f"vq_sb{rep}", bufs=1) as vqs,
                tc.tile_pool(name=f"vq_sc{rep}", bufs=2) as vsc,
                tc.tile_pool(name=f"vq_big{rep}", bufs=1) as vbg,
                tc.tile_pool(name=f"vq_sm{rep}", bufs=2) as vsm,
                tc.tile_pool(name=f"vq_ps{rep}", bufs=2, space="PSUM") as vqp,
              ):
                cb0 = vqs.tile([128, K_CODES], BF16)
                cb1 = vqs.tile([128, K_CODES], BF16)
                cb2 = vqs.tile([2, K_CODES], BF16)
                # column-chunked loads so the first score matmuls start after
                # ~1/4 of the codebook transfer instead of all of it
                for cc in range(4):
                    cs_ = slice(cc * 2048, (cc + 1) * 2048)
                    nc.sync.dma_start(cb0[:, cs_], cbt16[0:128, cs_])
                    nc.sync.dma_start(cb1[:, cs_], cbt16[128:256, cs_])
                nc.sync.dma_start(cb2[:], cbt16[256:258, :])
                zt0 = vqs.tile([128, TOKS], BF16)
                zt1 = vqs.tile([128, TOKS], BF16)
                zt2 = vqs.tile([2, TOKS], BF16)
                nc.sync.dma_start(zt0[:], zet16[0:128, :])
                nc.sync.dma_start(zt1[:], zet16[128:256, :])
                nc.sync.dma_start(zt2[:], zet16[256:258, :])
                posb = vqs.tile([128, NT * D_MODEL], BF16)
                nc.sync.dma_start(
                    posb[:].rearrange("p (t d) -> p t d", d=D_MODEL),
                    pos2[:].rearrange("(t p) d -> p t d", p=128))

                for t in range(NT):
                    tsl = slice(t * 128, t * 128 + 128)
                    zea = vsm.tile([128, CAUG], F32, tag="zea", name=f"zea_{rep}_{t}")
                    nc.sync.dma_start(
                        zea[:], zeaug[t * 128:(t + 1) * 128, :])
                    sc16 = vsc.tile([128, K_CODES], F16, tag="sc16")
                    for qtr in range(8):
                        ps = vqp.tile([128, 1024], F32, tag="vq", name=f"vps{rep}_{t}_{qtr}")
                        for ch in range(2):
                            c0 = qtr * 1024 + ch * 512
                            o = ps[:, ch * 512:ch * 512 + 512]
                            nc.tensor.matmul(o, zt0[:, tsl], cb0[:, c0:c0 + 512],
                                             start=True, stop=False)
                            nc.tensor.matmul(o, zt1[:, tsl], cb1[:, c0:c0 + 512],
                                             start=False, stop=False)
                            nc.tensor.matmul(o, zt2[:, tsl], cb2[:, c0:c0 + 512],
                                             start=False, stop=True)
                        nc.scalar.copy(sc16[:, qtr * 1024:qtr * 1024 + 1024], ps[:])
                    # depth-1 max tree: reduce the two 4096-halves (f16, 2x DVE
                    # mode), then max8/max_index scan only 4096 positions. Any
                    # f16-top-4 code's position lands in the top-4 positions of
                    # the reduced array, so rescoring top-4 positions x both
                    # halves (8 exact rescores) covers the true argmin.
                    red = vbg.tile([128, 4096], F16, tag="red")
                    nc.vector.tensor_tensor(out=red[:], in0=sc16[:, 0:4096],
                                            in1=sc16[:, 4096:8192], op=ALU.max)
                    m8 = vsm.tile([128, 8], F16, tag="m8")
                    i8 = vsm.tile([128, 8], U32, tag="i8")
                    nc.vector.max(out=m8[:], in_=red[:])
                    nc.vector.max_index(out=i8[:], in_max=m8[:], in_values=red[:])
                    i4f = vsm.tile([128, 4], F32, tag="i4f")
                    nc.gpsimd.tensor_copy(i4f[:], i8[:, :4])  # u32 -> f32
                    c8f = vsm.tile([128, 8], F32, tag="c8f")  # candidate codes
                    nc.gpsimd.tensor_copy(c8f[:, :4], i4f[:])
                    nc.gpsimd.tensor_scalar(c8f[:, 4:8], i4f[:], 4096.0, None,
                                            op0=ALU.add)
                    c4u = vsm.tile([128, 4], U32, tag="c4u")
                    nc.gpsimd.tensor_copy(c4u[:], i4f[:])  # f32 -> u32 (pair idx)
                    # exact rescore: 4 paired-row gathers (rows p and p+4096
                    # stored adjacently in cbpair), 8 affine rescores
                    cs = vsm.tile([128, NCAND], F32, tag="cs")
                    for k in range(4):
                        gat = vsm.tile([128, 2 * CAUG], F32, tag="gat",
                                       name=f"gat_{rep}_{t}_{k}")
                        nc.gpsimd.indirect_dma_start(
                            out=gat[:], out_offset=None, in_=cbpair[:],
                            in_offset=bass.IndirectOffsetOnAxis(ap=c4u[:, k:k + 1], axis=0))
                        for sub in range(2):
                            scr = vsm.tile([128, CAUG], BF16, tag="scr",
                                           name=f"scr_{rep}_{t}_{k}_{sub}")
                            nc.vector.affine_mul_reduce(
                                out=scr[:], accum_out=cs[:, k + 4 * sub:k + 4 * sub + 1],
                                in0=zea[:],
                                in1=gat[:, sub * CAUG:(sub + 1) * CAUG],
                                scale=1.0, bias=0.0)
                    # pick best (ties -> smallest code index, matching argmin)
                    best = vsm.tile([128, 1], F32, tag="best")
                    nc.vector.reduce_max(best[:], cs[:], axis=mybir.AxisListType.X)
                    eq = vsm.tile([128, NCAND], F32, tag="eq")
                    nc.gpsimd.tensor_scalar(eq[:], cs[:], best[:, :1], None, op0=ALU.is_ge)
                    msk = vsm.tile([128, NCAND], F32, tag="msk")
                    nc.gpsimd.tensor_scalar(msk[:], eq[:], -1e9, 1e9,
                                            op0=ALU.mult, op1=ALU.add)
                    nc.gpsimd.tensor_tensor(out=msk[:], in0=msk[:], in1=c8f[:, :NCAND], op=ALU.add)
                    bidxf = vsm.tile([128, 1], F32, tag="bidxf")
                    nc.vector.tensor_reduce(bidxf[:], msk[:], axis=mybir.AxisListType.X,
                                            op=ALU.min)
                    bidx = vsm.tile([128, 1], U32, tag="bidx")
                    nc.gpsimd.tensor_copy(bidx[:], bidxf[:])  # f32 -> u32
                    zq = vsm.tile([128, CAUG], F32, tag="zq")
                    nc.gpsimd.indirect_dma_start(
                        out=zq[:], out_offset=None, in_=cbaug[:],
                        in_offset=bass.IndirectOffsetOnAxis(ap=bidx[:, :1], axis=0))
                    # x = zq + pos (both SBUF -> Pool is legal and idle here)
                    nc.gpsimd.tensor_tensor(
                        out=xall[:, t * 256:(t + 1) * 256], in0=zq[:, :256],
                        in1=posb[:, t * 256:(t + 1) * 256], op=ALU.add)
                    if do_blocks and t in (7, 15):
                        # block-0 LN1 + transposes for this batch half run on
                        # ACT/Pool/PE while DVE drains the remaining VQ tiles
                        bh = t // 8
                        st0 = _ln_stats(nc, sm, xall, "pre", "ln1", bh)
                        _ln_apply(nc, xall, h16_0, st0, bh, pool_only=True)
                        _transpose_to(nc, tc, ident16, hT_0, h16_0,
                                      f"hT0_{rep}_{bh}", tt0=bh * 8, tt1=bh * 8 + 8,
                                      act_only=True)
                    if do_blocks and t == 7:
                        # block-0 qkv (batch half 0) + v quarters 0,1 overlap
                        # the VQ DVE drain of tiles 8-15 (qp: 4 PSUM banks)
                        with tc.tile_pool(name=f"qkv0_ps_{rep}", bufs=2,
                                          space="PSUM") as qp0:
                            _qkv_half(nc, qp0, qkT_0, hT_0, wqk_sb0, bqkc0,
                                      vaug, wv_sb0, f"{rep}_pre", 0, act_only=True)

            # ================= Phase 2: transformer blocks =================
            pending = {}
            with (
                tc.tile_pool(name=f"blk_w{rep}", bufs=2) as bw,
                tc.tile_pool(name=f"blk_act{rep}", bufs=1) as ba,
                tc.tile_pool(name=f"blk_probs{rep}", bufs=8) as bpr,
                tc.tile_pool(name=f"blk_sm{rep}", bufs=4) as bsm,
                tc.tile_pool(name=f"blk_nrm{rep}", bufs=2) as bnr,
            ):
                for blk0 in (range(N_BLOCKS) if do_blocks else []):
                    blk = blk0  # numeric index for DRAM slicing
                    nm = f"{rep}_{blk0}"
                    # ---- load weights for this block
                    wqk_sb = [bw.tile([128, 512], BF16, tag=f"wqk{d}", name=f"wqk_{nm}_{d}") for d in range(2)]
                    wv_sb = [bw.tile([128, 256], BF16, tag=f"wv{d}", name=f"wv_{nm}_{d}") for d in range(2)]
                    wo_sb = [bw.tile([128, 256], BF16, tag=f"wo{d}", name=f"wo_{nm}_{d}") for d in range(2)]
                    w1_sb = [bw.tile([128, DFF], BF16, tag=f"w1{d}", name=f"w1_{nm}_{d}") for d in range(2)]
                    w2c = bw.tile([128, 8 * 256], BF16, tag="w2c", name=f"w2c_{nm}")
                    w2_sb = [w2c[:, d * 256:(d + 1) * 256] for d in range(8)]
                    skip_qkvw = blk == 0 and do_vq
                    for d in range(2):
                        if not skip_qkvw:
                            nc.sync.dma_start(wqk_sb[d][:], wqk[blk, d * 128:(d + 1) * 128, :])
                            nc.sync.dma_start(wv_sb[d][:], wv[blk, d * 128:(d + 1) * 128, :])
                        nc.sync.dma_start(wo_sb[d][:], wo[blk, d * 128:(d + 1) * 128, :])
                        nc.sync.dma_start(w1_sb[d][:], w1[blk, d * 128:(d + 1) * 128, :])
                    nc.sync.dma_start(
                        w2c[:].rearrange("p (d m) -> p d m", m=256),
                        w2[blk].rearrange("(d p) m -> p d m", p=128))
                    # consolidated bias loads: one DMA each instead of 4+8 tiny ones
                    bqkc = bw.tile([128, 4], F32, tag="bqkc", name=f"bqkc_{nm}")
                    if not skip_qkvw:
                        nc.sync.dma_start(
                            bqkc[:].rearrange("p (m o) -> p m o", o=1),
                            bqk[blk].rearrange("(m p) o -> p m o", p=128))
                    b1c = bw.tile([128, 8], F32, tag="b1c", name=f"b1c_{nm}")
                    nc.sync.dma_start(
                        b1c[:].rearrange("p (m o) -> p m o", o=1),
                        b1[blk].rearrange("(m p) o -> p m o", p=128))
                    bo_sb = bw.tile([1, 256], BF16, tag="bo", name=f"bo_{nm}")
                    b2_sb = bw.tile([1, 256], BF16, tag="b2", name=f"b2_{nm}")
                    nc.sync.dma_start(bo_sb[:], bo16[blk])
                    nc.sync.dma_start(b2_sb[:], b216[blk])

                    # ---- LN1 -> h16 (bf16) ; gamma/beta folded into weights
                    if blk == 0 and do_vq:
                        hT = hT_0
                    elif blk in pending:
                        h16, hT = pending.pop(blk)
                        st1 = _ln_stats(nc, bsm, xall, nm, "ln1", 1)
                        _ln_apply(nc, xall, h16, st1, 1)
                        _transpose_to(nc, tc, ident16, hT, h16, f"hT{nm}b",
                                      tt0=8, tt1=16)
                    else:
                        h16 = ba.tile([128, NT * 256], BF16, tag="h16", name=f"h16_{nm}")
                        _layernorm(nc, tc, bsm, xall, h16, nm, "ln1")
                        # ---- hT via PE transposes
                        hT = [ba.tile([128, TOKS], BF16, tag=f"hT{d}", name=f"hT_{nm}_{d}") for d in range(2)]
                        _transpose_to(nc, tc, ident16, hT, h16, f"hT{nm}")

                    # ---- qkT (feature-major) and vaug (token-major, [v|1] cols)
                    if blk == 0 and do_vq:
                        qkT = qkT_0
                        wqk_u, wv_u, bqk_u = wqk_sb0, wv_sb0, bqkc0
                    else:
                        qkT = [ba.tile([128, TOKS], BF16, tag=f"qkT{m}", name=f"qkT_{nm}_{m}") for m in range(4)]
                        wqk_u, wv_u, bqk_u = wqk_sb, wv_sb, bqkc
                    with tc.tile_pool(name=f"qkv_ps_{rep}_{blk}", bufs=2, space="PSUM") as qp:
                        for half in range(2):
                            if blk == 0 and do_vq and half == 0:
                                continue  # emitted inside the VQ scope
                            _qkv_half(nc, qp, qkT, hT, wqk_u, bqk_u,
                                      vaug, wv_u, nm, half)

                    # ---- attention (batch-major) fused with S3 + LN2 stats:
                    # after batch b's two head-groups finish, its S3 quarters
                    # and LN2 stats run while batch b+1's attention proceeds.
                    oT16 = [ba.tile([128, SEQLEN], BF16, tag=f"oT{i}", name=f"oT_{nm}_{i}")
                            for i in range(4)]  # (b, g) -> [4heads*32, 1024 q]
                    h2 = ba.tile([128, NT * 256], BF16, tag="h16", name=f"h2_{nm}")
                    ln2st = []

                    def _emit_s3(b, s3pool):
                        for g4 in (2 * b, 2 * b + 1):
                            po = s3pool.tile([128, 1024], F32, tag="po", name=f"po_{nm}_{g4}")
                            for j in range(4):
                                t = g4 * 4 + j
                                q = (t % TPB) * 128
                                o = po[:, j * 256:(j + 1) * 256]
                                for g in range(2):
                                    nc.tensor.matmul(o, oT16[b * 2 + g][:, q:q + 128],
                                                     wo_sb[g][:], start=(g == 0), stop=False)
                                nc.tensor.matmul(o, ones16[:, :128], bo_sb[:],
                                                 start=False, stop=True)
                            w = slice(g4 * 1024, (g4 + 1) * 1024)
                            nc.vector.tensor_tensor(
                                out=xall[:, w], in0=xall[:, w], in1=po[:], op=ALU.add)

                    if True:
                        with (
                            tc.tile_pool(name=f"attn_s1_{nm}", bufs=4, space="PSUM") as s1p,
                            tc.tile_pool(name=f"attn_acc_{nm}", bufs=1, space="PSUM") as accp,
                            tc.tile_pool(name=f"s3_ps_{rep}_{blk}", bufs=1, space="PSUM") as s3p,
                        ):
                            for b in range(2):
                                for g in range(2):
                                    _attn_group(nc, tc, bpr, bnr, qkT, vaug, mln16,
                                                oT16[b * 2 + g], nm, b, g, s1p, accp)
                                _emit_s3(b, s3p)
                                ln2st.append(_ln_stats(nc, bsm, xall, nm, "ln2", b))

                    # ---- LN2 apply -> h2 -> h2T
                    for b in range(2):
                        _ln_apply(nc, xall, h2, ln2st[b], b)
                    h2T = [ba.tile([128, TOKS], BF16, tag=f"hT{d}", name=f"h2T_{nm}_{d}") for d in range(2)]
                    _transpose_to(nc, tc, ident16, h2T, h2, f"h2T{nm}")

                    # ---- FFN: gT = gelu(W1T h2T + b1) ; x += gT.T @ W2 + b2
                    # hh-outer so FFN2's first token chunks unblock after 8 gelus
                    gT = [ba.tile([128, TOKS], BF16, tag=f"gT{p}", name=f"gT_{nm}_{p}") for p in range(8)]
                    with tc.tile_pool(name=f"ffn1_ps_{rep}_{blk}", bufs=3, space="PSUM") as f1p:
                        for hh in range(2):
                            for p in range(8):
                                pu = f1p.tile([128, 1024], F32, tag="pu", name=f"pu_{nm}_{p}_{hh}")
                                for ch in range(2):
                                    c0 = hh * 1024 + ch * 512
                                    o = pu[:, ch * 512:ch * 512 + 512]
                                    nc.tensor.matmul(o, w1_sb[0][:, p * 128:(p + 1) * 128],
                                                     h2T[0][:, c0:c0 + 512], start=True, stop=False)
                                    nc.tensor.matmul(o, w1_sb[1][:, p * 128:(p + 1) * 128],
                                                     h2T[1][:, c0:c0 + 512], start=False, stop=True)
                                nc.scalar.activation(gT[p][:, hh * 1024:(hh + 1) * 1024], pu[:],
                                                     AF.Gelu_apprx_tanh, bias=b1c[:, p:p + 1])
                    with tc.tile_pool(name=f"ffn2_ps_{rep}_{blk}", bufs=3, space="PSUM") as f2p:
                        for g4 in range(4):
                            pf = f2p.tile([128, 1024], F32, tag="pf", name=f"pf_{nm}_{g4}")
                            for j in range(4):
                                t = g4 * 4 + j
                                tsl = slice(t * 128, t * 128 + 128)
                                o = pf[:, j * 256:(j + 1) * 256]
                                for p in range(8):
                                    nc.tensor.matmul(o, gT[p][:, tsl],
                                                     w2c[:, p * 256:(p + 1) * 256],
                                                     start=(p == 0), stop=False)
                                nc.tensor.matmul(o, ones16[:, :128], b2_sb[:],
                                                 start=False, stop=True)
                            w = slice(g4 * 1024, (g4 + 1) * 1024)
                            nc.vector.tensor_tensor(
                                out=xall[:, w], in0=xall[:, w], in1=pf[:], op=ALU.add)


                # ================= Phase 3: output projection =================
                wout_sb = [bw.tile([128, D_PATCH], BF16, tag=f"wout{d}", name=f"wout_{rep}_{d}") for d in range(2)]
                for d in range(2):
                    nc.sync.dma_start(wout_sb[d][:], wout[d * 128:(d + 1) * 128, :])
                bout_sb = bw.tile([1, D_PATCH], BF16, tag="bout", name=f"bout_{rep}")
                nc.sync.dma_start(bout_sb[:], bout16[:])
                # transpose fp32 xall directly (2 cy/row), psum -> bf16 xT
                xT = [ba.tile([128, TOKS], BF16, tag=f"hT{d}", name=f"xT_{d}") for d in range(2)]
                _transpose_fp32(nc, tc, ident32, xT, xall, f"xTo_{rep}f", 0, NT)
                with (
                    tc.tile_pool(name=f"out_ps{rep}", bufs=4, space="PSUM") as op,
                    tc.tile_pool(name=f"out_st{rep}", bufs=2) as ost,
                ):
                    # stage 2 tiles per DMA: single-DMA issue overhead (~2us on
                    # the SP queue) otherwise serializes the whole tail
                    for tq in range(8):
                        ol4 = ost.tile([128, 2 * D_PATCH], F32, tag="ol4",
                                       name=f"ol4_{rep}_{tq}")
                        for j in range(2):
                            t = tq * 2 + j
                            tsl = slice(t * 128, t * 128 + 128)
                            pl = op.tile([128, 1024], F32, tag="pl", name=f"pl_{rep}_{t}")
                            for ch, w in ((0, 512), (1, 256)):
                                o = pl[:, ch * 512:ch * 512 + w]
                                nc.tensor.matmul(o, xT[0][:, tsl], wout_sb[0][:, ch * 512:ch * 512 + w],
                                                 start=True, stop=False)
                                nc.tensor.matmul(o, xT[1][:, tsl], wout_sb[1][:, ch * 512:ch * 512 + w],
                                                 start=False, stop=False)
                                nc.tensor.matmul(o, ones16[:, :128], bout_sb[:, ch * 512:ch * 512 + w],
                                                 start=False, stop=True)
                            dst = ol4[:, j * D_PATCH:(j + 1) * D_PATCH]
                            if t % 2 == 0:
                                nc.scalar.copy(dst, pl[:, :768])
                            else:
                                nc.vector.tensor_copy(dst, pl[:, :768])
                        nc.sync.dma_start(
                            logits[tq * 256:(tq + 1) * 256, :].rearrange(
                                "(t p) d -> p t d", p=128),
                            ol4[:].rearrange("p (t d) -> p t d", d=D_PATCH))

    nc.compile()
    return nc


def _qkv_half(nc, qp, qkT, hT, wqk_sb, bqkc, vaug, wv_sb, nm, half, act_only=False):
    """q/k projections for one batch half (512-wide PSUM chunks, 4 banks) and
    the V quarters of that half written into vaug's [v|1] layout."""
    c0h = half * 1024
    for m in range(4):
        for sub in range(2):
            pq = qp.tile([128, 512], F32, tag="pq", name=f"pq_{nm}_{m}_{half}_{sub}")
            c0 = c0h + sub * 512
            nc.tensor.matmul(pq[:], wqk_sb[0][:, m * 128:(m + 1) * 128],
                             hT[0][:, c0:c0 + 512], start=True, stop=False)
            nc.tensor.matmul(pq[:], wqk_sb[1][:, m * 128:(m + 1) * 128],
                             hT[1][:, c0:c0 + 512], start=False, stop=True)
            if (m + sub) % 2 == 0 and not act_only:
                nc.vector.tensor_scalar(qkT[m][:, c0:c0 + 512], pq[:],
                                        bqkc[:, m:m + 1], None, op0=ALU.add)
            else:
                nc.scalar.activation(qkT[m][:, c0:c0 + 512], pq[:],
                                     AF.Identity, bias=bqkc[:, m:m + 1])
    for g4 in (2 * half, 2 * half + 1):
        for s2 in range(2):
            pv = qp.tile([128, 512], F32, tag="pv", name=f"pv_{nm}_{g4}_{s2}")
            for j in range(2):
                t = g4 * 4 + s2 * 2 + j
                tsl = slice(t * 128, t * 128 + 128)
                o = pv[:, j * 256:(j + 1) * 256]
                nc.tensor.matmul(o, hT[0][:, tsl], wv_sb[0][:], start=True, stop=False)
                nc.tensor.matmul(o, hT[1][:, tsl], wv_sb[1][:], start=False, stop=True)
            # strided copy into vaug dims columns ([v|1] layout)
            base = g4 * 2048 + s2 * 1024
            vdst = vaug[:, base:base + 1024].rearrange(
                "p (t h e) -> p t h e", h=8, e=64)[:, :, :, 0:32]
            vsrc = pv[:].rearrange("p (t h e) -> p t h e", h=8, e=32)
            if (g4 + s2) % 2 == 0 and not act_only:
                nc.vector.tensor_copy(vdst, vsrc)
            else:
                nc.scalar.copy(vdst, vsrc)


def _transpose_fp32(nc, tc, ident32, dstT, xall, tag, tt0, tt1, bufs=4):
    """bf16 transposes of the fp32 residual (2 cy/row on PE)."""
    with tc.tile_pool(name=f"tp_ps_{tag}", bufs=bufs, space="PSUM") as tpp:
        for tt in range(tt0, tt1, 4):
            for d in range(2):
                pt = tpp.tile([128, 512], F32, tag="tp", name=f"tp_{tag}_{tt}_{d}")
                for j in range(4):
                    t = tt + j
                    nc.tensor.transpose(
                        pt[:, j * 128:(j + 1) * 128],
                        xall[:, t * 256 + d * 128: t * 256 + (d + 1) * 128],
                        ident32[:])
                if (tt // 4 + d) % 2 == 0:
                    nc.vector.tensor_copy(dstT[d][:, tt * 128:(tt + 4) * 128], pt[:])
                else:
                    nc.scalar.copy(dstT[d][:, tt * 128:(tt + 4) * 128], pt[:])


def _transpose_to(nc, tc, ident16, dstT, src, tag, tt0=0, tt1=NT, act_only=False, bufs=4):
    """dstT[d][:, t*128:(t+1)*128] = transpose(src[:, t*256+d*128 : ...]) via PE.
    4 transposes share one PSUM tile; single batched copy, DVE/ACT alternated."""
    with tc.tile_pool(name=f"tp_ps_{tag}", bufs=bufs, space="PSUM") as tpp:
        for tt in range(tt0, tt1, 4):
            for d in range(2):
                pt = tpp.tile([128, 512], BF16, tag="tp", name=f"tp_{tag}_{tt}_{d}")
                for j in range(4):
                    t = tt + j
                    nc.tensor.transpose(
                        pt[:, j * 128:(j + 1) * 128],
                        src[:, t * 256 + d * 128: t * 256 + (d + 1) * 128],
                        ident16[:])
                if (tt // 4 + d) % 2 == 0 and not act_only:
                    nc.vector.tensor_copy(dstT[d][:, tt * 128:(tt + 4) * 128], pt[:])
                else:
                    nc.scalar.copy(dstT[d][:, tt * 128:(tt + 4) * 128], pt[:])


def _ln_stats(nc, bsm, xall, blk, tag, bh):
    """bn stats + rstd for batch-half bh (tiles bh*8..bh*8+8).
    rstd computed as exp(-0.5 * ln(var + eps))."""
    NH = NT // 2
    t0 = bh * NH
    epsc = bsm.tile([128, 1], F32, tag="lneps", name=f"{tag}eps_{blk}_{bh}")
    nc.gpsimd.memset(epsc[:], LN_EPS)
    meancol = bsm.tile([128, NH], F32, tag="lnmean", name=f"{tag}mean_{blk}_{bh}")
    varcol = bsm.tile([128, NH], F32, tag="lnvar", name=f"{tag}var_{blk}_{bh}")
    rstdcol = bsm.tile([128, NH], F32, tag="lnrstd", name=f"{tag}rstd_{blk}_{bh}")
    lncol = bsm.tile([128, NH], F32, tag="lnln", name=f"{tag}ln_{blk}_{bh}")
    for i in range(NH):
        t = t0 + i
        stats = bsm.tile([128, 6], F32, tag="lnstats", name=f"{tag}stats_{blk}_{t}")
        mv = bsm.tile([128, 2], F32, tag="lnmv", name=f"{tag}mv_{blk}_{t}")
        nc.vector.bn_stats(stats[:], xall[:, t * 256:(t + 1) * 256])
        nc.vector.bn_aggr(mv[:], stats[:])
        nc.gpsimd.tensor_copy(meancol[:, i:i + 1], mv[:, 0:1])
        nc.gpsimd.tensor_copy(varcol[:, i:i + 1], mv[:, 1:2])
    nc.scalar.activation(lncol[:], varcol[:], AF.Ln, bias=epsc[:, :1])
    nc.scalar.activation(rstdcol[:], lncol[:], AF.Exp, scale=-0.5)
    return meancol, rstdcol


def _ln_apply(nc, xall, h16, st, bh, pool_only=False):
    """h16 tiles of batch-half bh = (x - mean) * rstd, Pool/DVE alternated."""
    meancol, rstdcol = st
    NH = NT // 2
    t0 = bh * NH
    for i in range(NH):
        t = t0 + i
        eng = nc.gpsimd if (pool_only or i % 2 == 0) else nc.vector
        eng.tensor_scalar(
            h16[:, t * 256:(t + 1) * 256], xall[:, t * 256:(t + 1) * 256],
            meancol[:, i:i + 1], rstdcol[:, i:i + 1],
            op0=ALU.subtract, op1=ALU.mult)


def _layernorm(nc, tc, bsm, xall, h16, blk, tag):
    for bh in range(2):
        st = _ln_stats(nc, bsm, xall, blk, tag, bh)
        _ln_apply(nc, xall, h16, st, bh)



def _attn_group(nc, tc, bpr, bnr, qkT, vaug, mln16, oT_out, blk, b, g, s1p, accp, exp_pat="AADAD"):
    """One (batch, 4-head-group): scoresT QK matmuls -> exp -> fused PV+rowsum
    matmuls ([v|1] 64-wide lhsT -> psum bands [d(32), s(32)] per head) ->
    normalize via base-shifted reciprocal + same-base mult.
    qc-outer with [128, 512] accumulators so attention holds only 6 PSUM banks.
    oT_out: [128 (4h x 32 dims), 1024 q] bf16."""
    name = f"{blk}_{b}_{g}"
    qt, kt = qkT[g], qkT[2 + g]
    tok0 = b * SEQLEN
    ci = 0
    for qc in range(2):
        qs = slice(qc * 512, qc * 512 + 512)
        # otp[half]: [d_h0(0:32), s_h0(32:64), d_h1(64:96), s_h1(96:128)] x 512 q
        otp = [accp.tile([128, 512], F32, tag=f"otp{half}",
                         name=f"otp_{name}_{qc}_{half}") for half in range(2)]
        for kk in range(8):
            vbase = (b * TPB + kk) * 512 + g * 256
            for half in range(2):
                for hh in range(2):
                    h = half * 2 + hh
                    # fine-grained per-head [128, 512] chunk: QK -> exp -> PV.
                    ps = s1p.tile([128, 512], F32, tag="s1",
                                  name=f"s1_{name}_{qc}_{kk}_{half}_{hh}")
                    nc.tensor.matmul(
                        ps[:],
                        kt[32 * h:32 * h + 32, tok0 + kk * 128: tok0 + (kk + 1) * 128],
                        qt[32 * h:32 * h + 32, tok0 + qc * 512: tok0 + (qc + 1) * 512],
                        start=True, stop=True, tile_position=(32 * h, 0))
                    pr = bpr.tile([128, 512], BF16, tag="probs",
                                  name=f"pr_{name}_{qc}_{kk}_{half}_{hh}")
                    # probs scaled by 1/16 (exp(s)-ln16) so f16 sums stay in
                    # range; the d/s ratio is scale-invariant.
                    eng = exp_pat[ci % len(exp_pat)]
                    ci += 1
                    if eng == "A":
                        nc.scalar.activation(pr[:], ps[:], AF.Exp, bias=mln16[:, :1])
                    else:
                        with nc.allow_low_precision(reason="approx softmax exp"):
                            nc.vector.tensor_scalar(
                                pr.bitcast(mybir.dt.uint16)[:], ps[:],
                                184.66496, 15738.5, op0=ALU.mult, op1=ALU.add)
                    # fused PV+rowsum: lhsT = [v_h(32) | ones(32)]
                    nc.tensor.matmul(otp[half][hh * 64:(hh + 1) * 64, :],
                                     vaug[:, vbase + h * 64: vbase + (h + 1) * 64],
                                     pr[:], start=(kk == 0), stop=(kk == 7),
                                     tile_position=(0, 64 * hh))
        # normalize this qc window: oT[h band] = d_h * (1 / s_h)
        for half in range(2):
            sb_acc = bnr.tile([128, 512], F16, tag=f"sbac{half}",
                              name=f"sbac_{name}_{qc}_{half}")
            nc.scalar.copy(sb_acc[:], otp[half][:])
            recS = bnr.tile([128, 512], F16, tag=f"recS{half}",
                            name=f"recS_{name}_{qc}_{half}")
            with nc.allow_low_precision(reason="softmax recip consumed by bf16 probs"):
                nc.vector.reciprocal(recS[0:32, :], sb_acc[32:64, :])
                nc.vector.reciprocal(recS[64:96, :], sb_acc[96:128, :])
            for hh in range(2):
                h = half * 2 + hh
                nc.vector.tensor_tensor(
                    out=oT_out[32 * h:32 * h + 32, qs], in0=sb_acc[64 * hh:64 * hh + 32, :],
                    in1=recS[64 * hh:64 * hh + 32, :], op=ALU.mult)



def _prep_inputs(inputs):
    """Host-side prep: shard, transpose, fold LN scales, cast."""
    ze = np.asarray(inputs["ze"], np.float32)
    cb = np.asarray(inputs["codebook"], np.float32)
    pos = np.asarray(inputs["pos_emb"], np.float32)
    n2 = (cb * cb).sum(-1)
    n2h = n2.astype(ml_dtypes.bfloat16).astype(np.float32)
    n2l = (n2 - n2h).astype(ml_dtypes.bfloat16).astype(np.float32)

    cbt = np.empty((258, K_CODES), np.float32)
    cbt[:256] = cb.T
    cbt[256] = -n2h
    cbt[257] = -n2l
    cbt16 = _bf16(cbt)
    cbaug = np.zeros((K_CODES, CAUG), np.float32)
    cbaug[:, :256] = cb
    cbaug[:, 256] = n2
    cbpair = np.concatenate([cbaug[:K_CODES // 2], cbaug[K_CODES // 2:]], axis=1)
    pos2 = _bf16(np.concatenate([pos] * B_PER_CORE, axis=0))

    shared = {"cbt16": cbt16, "cbaug": cbaug, "cbpair": cbpair, "pos2": pos2}

    sq = 1.0 / np.sqrt(np.float32(D_MODEL // N_HEADS))
    wqk_l, bqk_l, wv_l, wo_l, bo_l = [], [], [], [], []
    w1_l, b1_l, w2_l, b2_l = [], [], [], []
    for i in range(N_BLOCKS):
        Wqkv = np.asarray(inputs["Wqkv"][i], np.float32)
        bqkv = np.asarray(inputs["bqkv"][i], np.float32)
        s1v = np.asarray(inputs["ln1_s"][i], np.float32)
        b1v = np.asarray(inputs["ln1_b"][i], np.float32)
        Wf = s1v[:, None] * Wqkv
        bf = bqkv + b1v @ Wqkv
        Wf[:, :256] *= sq
        bf[:256] *= sq
        wqk_l.append(_bf16(Wf[:, :512]))
        bqk_l.append(bf[:512].astype(np.float32)[:, None])
        wv_l.append(_bf16(Wf[:, 512:]))
        Wo = np.asarray(inputs["Wo"][i], np.float32)
        wo_l.append(_bf16(Wo))
        # v bias folded into output-proj bias: softmax-weighted mean of a
        # constant is the constant, so attn(v + bv) @ Wo = attn(v) @ Wo + bv @ Wo
        bv = bf[512:]
        bo_l.append(_bf16((np.asarray(inputs["bo"][i], np.float32) + bv @ Wo)[None, :]))
        W1 = np.asarray(inputs["W1"][i], np.float32)
        s2v = np.asarray(inputs["ln2_s"][i], np.float32)
        b2v = np.asarray(inputs["ln2_b"][i], np.float32)
        w1_l.append(_bf16(s2v[:, None] * W1))
        b1_l.append((np.asarray(inputs["b1"][i], np.float32) + b2v @ W1).astype(np.float32)[:, None])
        w2_l.append(_bf16(inputs["W2"][i]))
        b2_l.append(_bf16(np.asarray(inputs["b2"][i], np.float32)[None, :]))
    shared.update({
        "wqk": np.stack(wqk_l), "bqk": np.stack(bqk_l),
        "wv": np.stack(wv_l),
        "wo": np.stack(wo_l), "bo16": np.stack(bo_l),
        "w1": np.stack(w1_l), "b1": np.stack(b1_l),
        "w2": np.stack(w2_l), "b216": np.stack(b2_l),
        "wout": _bf16(inputs["Wout"]),
        "bout16": _bf16(np.asarray(inputs["bout"], np.float32)[None, :]),
    })

    in_maps = []
    for c in range(N_CORES):
        zec = ze[c * B_PER_CORE:(c + 1) * B_PER_CORE].reshape(TOKS, D_MODEL)
        zet = np.empty((258, TOKS), np.float32)
        zet[:256] = (2.0 * zec).T
        zet[256:258] = 1.0
        zeaug = np.zeros((TOKS, CAUG), np.float32)
        zeaug[:, :256] = 2.0 * zec
        zeaug[:, 256] = -1.0
        in_maps.append({**shared, "zet16": _bf16(zet), "zeaug": zeaug})
    return in_maps


def kernel(**inputs) -> np.ndarray:
    if "nc" not in _BUILD_CACHE:
        _BUILD_CACHE["nc"] = build_nc("full", reps=1)
    nc = _BUILD_CACHE["nc"]
    in_maps = _prep_inputs(inputs)
    res = bass_utils.run_bass_kernel_spmd(nc, in_maps, core_ids=list(range(N_CORES)))
    out = np.stack([res.results[c]["logits"] for c in range(N_CORES)])
    return out.reshape(BATCH, SEQLEN, D_PATCH)
